# revision 2
# baseline (speedup 1.0000x reference)
"""Distributed causal MHA + RoPE kernel for 8 TRN2 NeuronCores (raw Bass).

Rewrite of the staged baseline with a qt-major software pipeline:

  - 8 cores = 2 batches x 4 head-groups (4 heads each); each core emits a
    partial out-projection (its heads' rows of Wout) in bf16; host sums.
  - Per q-block (512 rows) and head pair: S^T tiles via K=64 matmuls with
    row-group packing (head0 rows 0-63, head1 rows 64-127 issued
    back-to-back so they run concurrently in disjoint PE row groups),
    exp on ACT over [128,1024] spanning both heads' tiles (psum f32,
    double-buffered in 4 banks), causal diag mask on DVE, PV accumulated
    per head into dedicated psum banks with a ones-column for the
    softmax denominators.
  - RoPE: qk*cos + swap(qk)*sign_sin where swap is 4 SBUF->SBUF DMA
    block copies and sign_sin has the -/+ pattern baked in host-side;
    3 big DVE ops per (block, chunk).
  - Denominator reciprocals via DVE stream-transpose trick: transpose
    [32,512] so the 512 denoms land 16-per-partition, reciprocal on the
    strided view, transpose back; broadcast via DRAM round-trip DMA.
  - Projection (A), V (B) and out-projection (D) matmul chains are
    interleaved into the attention stream and borrow the PV psum banks
    of the opposite head-pair while those are idle.
"""

import numpy as np

B, T, D, H, DH = 2, 2048, 1024, 16, 64
HPC = 4
NG = 4
TQ = 512
NQT = T // TQ      # 4
NKT = T // 128     # 16
KC = D // 128      # 8


def _build_nc(causal: bool):
    import concourse.bass as bass
    import concourse.mybir as mybir
    from contextlib import ExitStack

    dt = mybir.dt
    f32, bf16 = dt.float32, dt.bfloat16
    AF = mybir.ActivationFunctionType
    nc = bass.Bass()

    xT = nc.declare_dram_parameter("xT", [D, T], bf16, isOutput=False)
    wqk = nc.declare_dram_parameter("wqk", [D, 512], bf16, isOutput=False)
    wv = nc.declare_dram_parameter("wv", [D, 256], bf16, isOutput=False)
    wo = nc.declare_dram_parameter("wo", [256, D], bf16, isOutput=False)
    cosr = nc.declare_dram_parameter("cosr", [128, T], bf16, isOutput=False)
    sinsg = nc.declare_dram_parameter("sinsg", [128, T], bf16, isOutput=False)
    dm2 = nc.declare_dram_parameter("dm2", [128, 4096], bf16, isOutput=False)
    out = nc.declare_dram_parameter("out", [T, D], bf16, isOutput=True)
    den_dram = nc.dram_tensor("den_dram", [2, 1024], f32)
    rec_dram = nc.dram_tensor("rec_dram", [2, 1024], f32)

    ctx = ExitStack()
    with ctx:
        sb = lambda name, shape, dtype: ctx.enter_context(
            nc.sbuf_tensor(name, shape, dtype))

        xt_sb = sb("xt_sb", [128, KC, T], bf16)
        wqk_sb = sb("wqk_sb", [128, KC, 512], bf16)
        wv_sb = sb("wv_sb", [128, KC, 256], bf16)
        wo_sb = sb("wo_sb", [128, 2, D], bf16)
        cos_sb = sb("cos_sb", [128, T], bf16)
        sin_sb = sb("sin_sb", [128, T], bf16)
        dm_sb = sb("dm_sb", [128, 4, 1024], bf16)
        qk_sb = sb("qk_sb", [128, 4, T], bf16)
        qsw_sb = sb("qsw_sb", [128, 4, T], bf16)
        qkr_sb = sb("qkr_sb", [128, 2, T], bf16)
        # k stationary, zero-padded to K=128: kz[:,hh,mk,:] holds head hh's
        # rotated k rows in its own 64-partition half, zeros elsewhere, so
        # every S matmul is a full-array (128,128)-tile op - no PE tiling
        # mode switches mid-stream.
        kz_sb = sb("kz_sb", [128, 2, 2, T], bf16)
        rtmp_sb = sb("rtmp_sb", [128, 2, TQ], bf16)
        vp_sb = sb("vp_sb", [128, NKT, HPC * 65], bf16)
        p_sb = sb("p_sb", [128, 2, 1024], bf16)
        at_sb = sb("at_sb", [128, 2, T], bf16)
        ob_sb = sb("ob_sb", [128, 2, 512], bf16)
        den_sb = sb("den_sb", [1, 2, 512], f32)
        rsq_sb = sb("rsq_sb", [128, 8], f32)
        rsq2_sb = sb("rsq2_sb", [128, 8], f32)
        rb_sb = sb("rb_sb", [128, 2, 512], f32)

        pS = ctx.enter_context(nc.psum_tensor("pS", [128, 2048], f32))
        P = [ctx.enter_context(nc.psum_tensor(f"P{i}", [128, 512], f32))
             for i in range(4)]

        in_sems = ["iw", "ic", "isn", "iv", "io", "idm",
                   "ix0", "ix1", "ix2", "ix3"]
        sw_sems = [f"sw{i}" for i in range(16)]
        sem_names = (["pe", "act", "dve", "r1", "r2", "ro", "rb",
                      "od0", "od1"] + in_sems + sw_sems)
        sems = {n: ctx.enter_context(nc.semaphore(f"s_{n}")) for n in sem_names}
        block = ctx.enter_context(nc.Block())

        # ---------- schedule machinery ----------
        sched = []
        cnt = {n: 0 for n in sem_names}
        last_wait = {}

        def wait(eng, sem, val):
            if val <= 0:
                return
            key = (eng, sem)
            if last_wait.get(key, -1) >= val:
                return
            last_wait[key] = val
            sched.append((eng, lambda e, s=sems[sem], v=val: e.wait_ge(s, v)))

        def emit(eng, fn, inc=None, inc_by=1):
            if inc is None:
                sched.append((eng, fn))
            else:
                s = sems[inc]
                sched.append((eng, lambda e, f=fn, ss=s, ib=inc_by:
                              f(e).then_inc(ss, ib)))
                cnt[inc] += inc_by

        # ---------- state tracking ----------
        rope_done = {}       # (tb, m) -> dve cnt
        vp_done = {}         # tt -> dve cnt
        exp_done = {}        # buf -> act cnt
        mask_done = {}       # buf -> dve cnt
        pvread_done = {}     # buf -> pe cnt (PV pair that read p_sb[buf])
        bank_free = {i: None for i in range(4)}   # bank -> (sem, cnt)
        norm_done = {}       # (qt, hp) -> dve cnt
        od_cnt = {0: 0, 1: 0}
        ob_slot = [0]
        swap_grp = [0]

        def wait_bank(bank):
            g = bank_free[bank]
            if g is not None:
                wait("tensor", g[0], g[1])

        # ---------- input DMAs ----------
        def dma_in(sem, dst, src):
            emit("sync", lambda e, d=dst, s=src: e.dma_start(out=d, in_=s),
                 inc=sem, inc_by=16)

        xre = xT.rearrange("(c p) t -> p c t", p=128)
        dma_in("iw", wqk_sb[:], wqk.rearrange("(c p) m -> p c m", p=128))
        dma_in("ix0", xt_sb[:, :, 0:TQ], xre[:, :, 0:TQ])
        dma_in("ic", cos_sb[:], cosr[:])
        dma_in("isn", sin_sb[:], sinsg[:])
        dma_in("iv", wv_sb[:], wv.rearrange("(c p) m -> p c m", p=128))
        dma_in("ix1", xt_sb[:, :, TQ:2 * TQ], xre[:, :, TQ:2 * TQ])
        dma_in("io", wo_sb[:], wo.rearrange("(h p) n -> p h n", p=128))
        dma_in("idm", dm_sb[:], dm2.rearrange("p (r n) -> p r n", r=4))
        dma_in("ix2", xt_sb[:, :, 2 * TQ:3 * TQ], xre[:, :, 2 * TQ:3 * TQ])
        dma_in("ix3", xt_sb[:, :, 3 * TQ:4 * TQ], xre[:, :, 3 * TQ:4 * TQ])

        # ---------- memsets ----------
        vpv = vp_sb.rearrange("p n (h m) -> p n h m", m=65)
        emit("vector", lambda e: nc.vector.memset(vpv[:, :, :, 64:65], 1.0),
             inc="dve")
        emit("vector", lambda e: nc.vector.memset(kz_sb[:], 0.0), inc="dve")
        memset_done = cnt["dve"]

        # ---------- chains ----------
        def a_chain(tb, m, bank):
            """qkT projection chunk m of t-block tb + rope pipeline."""
            sl = slice(tb * TQ, (tb + 1) * TQ)
            wait("tensor", "iw", 16)
            wait("tensor", f"ix{tb}", 16)
            wait_bank(bank)
            for c in range(KC):
                emit("tensor",
                     lambda e, cc=c, mm=m, s=sl: nc.tensor.matmul(
                         P[bank][:], wqk_sb[:, cc, mm * 128:(mm + 1) * 128],
                         xt_sb[:, cc, s],
                         start=(cc == 0), stop=(cc == KC - 1)),
                     inc="pe" if c == KC - 1 else None)
            pe_after = cnt["pe"]
            wait("scalar", "pe", pe_after)
            emit("scalar",
                 lambda e, mm=m, s=sl: nc.scalar.copy(
                     qk_sb[:, mm, s], P[bank][:]),
                 inc="act")
            copy_cnt = cnt["act"]
            bank_free[bank] = ("act", copy_cnt)
            # swap DMAs
            g = swap_grp[0]
            swap_grp[0] += 1
            gsem = f"sw{g}"
            wait("sync", "act", copy_cnt)
            for dst0, src0 in ((0, 32), (32, 0), (64, 96), (96, 64)):
                emit("sync",
                     lambda e, d0=dst0, s0=src0, mm=m, s=sl: e.dma_start(
                         out=qsw_sb[d0:d0 + 32, mm, s],
                         in_=qk_sb[s0:s0 + 32, mm, s]),
                     inc=gsem, inc_by=16)
            # rope on DVE
            wait("vector", gsem, 64)
            wait("vector", "ic", 16)
            wait("vector", "isn", 16)
            emit("vector",
                 lambda e, mm=m, s=sl: nc.vector.tensor_mul(
                     rtmp_sb[:, 0, :], qk_sb[:, mm, s], cos_sb[:, s]))
            emit("vector",
                 lambda e, mm=m, s=sl: nc.vector.tensor_mul(
                     rtmp_sb[:, 1, :], qsw_sb[:, mm, s], sin_sb[:, s]))
            if m < 2:
                emit("vector",
                     lambda e, mm=m, s=sl: nc.vector.tensor_add(
                         qkr_sb[:, mm, s], rtmp_sb[:, 0, :], rtmp_sb[:, 1, :]),
                     inc="dve")
            else:
                emit("vector",
                     lambda e, mk=m - 2, s=sl: nc.vector.tensor_add(
                         kz_sb[0:64, 0, mk, s], rtmp_sb[0:64, 0, :],
                         rtmp_sb[0:64, 1, :]))
                emit("vector",
                     lambda e, mk=m - 2, s=sl: nc.vector.tensor_add(
                         kz_sb[64:128, 1, mk, s], rtmp_sb[64:128, 0, :],
                         rtmp_sb[64:128, 1, :]),
                     inc="dve")
            rope_done[(tb, m)] = cnt["dve"]

        def b_chain(tb, tt, bank):
            """V natural chunk for ktile tt (128 rows)."""
            wait("tensor", "iv", 16)
            wait("tensor", f"ix{tb}", 16)
            wait_bank(bank)
            for c in range(KC):
                emit("tensor",
                     lambda e, cc=c, t2=tt: nc.tensor.matmul(
                         P[bank][:, 0:256],
                         xt_sb[:, cc, t2 * 128:(t2 + 1) * 128],
                         wv_sb[:, cc, :],
                         start=(cc == 0), stop=(cc == KC - 1)),
                     inc="pe" if c == KC - 1 else None)
            wait("vector", "pe", cnt["pe"])
            wait("vector", "dve", memset_done)
            emit("vector",
                 lambda e, t2=tt: nc.vector.tensor_copy(
                     vpv[:, t2, :, 0:64],
                     P[bank][:, 0:256].rearrange("p (h m) -> p h m", m=64)),
                 inc="dve")
            vp_done[tt] = cnt["dve"]
            bank_free[bank] = ("dve", cnt["dve"])

        def d_chain(qt, tq, n, bank):
            """out-projection tile: ttile tq (128 rows), n-chunk (512 cols)."""
            wait("tensor", "io", 16)
            wait("tensor", "dve", norm_done[(qt, 0)])
            wait("tensor", "dve", norm_done[(qt, 1)])
            wait_bank(bank)
            for hp in range(2):
                emit("tensor",
                     lambda e, h2=hp, t2=tq, n2=n: nc.tensor.matmul(
                         P[bank][:],
                         at_sb[:, h2, t2 * 128:(t2 + 1) * 128],
                         wo_sb[:, h2, n2 * 512:(n2 + 1) * 512],
                         start=(h2 == 0), stop=(h2 == 1)),
                     inc="pe" if hp == 1 else None)
            wait("vector", "pe", cnt["pe"])
            slot = ob_slot[0]
            ob_slot[0] ^= 1
            # ob slot reuse: wait for previous out-DMA from this slot
            wait("vector", f"od{slot}", od_cnt[slot])
            emit("vector",
                 lambda e, s=slot: nc.vector.tensor_copy(
                     ob_sb[:, s, :], P[bank][:]),
                 inc="dve")
            bank_free[bank] = ("dve", cnt["dve"])
            wait("sync", "dve", cnt["dve"])
            emit("sync",
                 lambda e, t2=tq, n2=n, s=slot: e.dma_start(
                     out=out[t2 * 128:(t2 + 1) * 128, n2 * 512:(n2 + 1) * 512],
                     in_=ob_sb[:, s, :]),
                 inc=f"od{slot}", inc_by=16)
            od_cnt[slot] = cnt[f"od{slot}"]

        # ---------- prologue ----------
        a_chain(0, 0, 2)
        a_chain(0, 2, 3)
        b_chain(0, 0, 2)
        b_chain(0, 1, 3)
        a_chain(0, 1, 2)
        a_chain(0, 3, 3)
        b_chain(0, 2, 2)
        b_chain(0, 3, 3)

        # ---------- attention + interleaved chains ----------
        scale = 0.125

        def c_block(qt, hp, interleave):
            """Attention for (qt, hp). interleave: list of chain thunks."""
            nkt = 4 * (qt + 1) if causal else NKT
            il = list(interleave)
            for kj in range(nkt):
                buf = kj % 2
                # --- S pair ---
                if exp_done.get(buf) is not None:
                    wait("tensor", "act", exp_done[buf])
                wait("tensor", "dve", rope_done[(kj // 4, 2 + hp)])
                wait("tensor", "dve", rope_done[(qt, hp)])
                for hh in range(2):
                    emit("tensor",
                         lambda e, h2=hh, k2=kj, q2=qt, p2=hp, b2=buf:
                         nc.tensor.matmul(
                             pS[:, b2 * 1024 + h2 * 512:
                                b2 * 1024 + (h2 + 1) * 512],
                             kz_sb[:, h2, p2, k2 * 128:(k2 + 1) * 128],
                             qkr_sb[:, p2, q2 * TQ:(q2 + 1) * TQ],
                             start=True, stop=True),
                         inc="pe" if hh == 1 else None)
                s_cnt = cnt["pe"]
                # --- exp ---
                wait("scalar", "pe", s_cnt)
                if mask_done.get(buf) is not None:
                    wait("scalar", "dve", mask_done[buf])
                if pvread_done.get(buf) is not None:
                    wait("scalar", "pe", pvread_done[buf])
                emit("scalar",
                     lambda e, b2=buf: nc.scalar.activation(
                         p_sb[:, b2, :], pS[:, b2 * 1024:(b2 + 1) * 1024],
                         AF.Exp, scale=scale),
                     inc="act")
                exp_done[buf] = cnt["act"]
                # --- diag mask ---
                r = kj - 4 * qt
                diag = causal and r >= 0
                if diag:
                    wait("vector", "act", cnt["act"])
                    wait("vector", "idm", 16)
                    emit("vector",
                         lambda e, b2=buf, r2=r: nc.vector.tensor_mul(
                             p_sb[:, b2, :], p_sb[:, b2, :], dm_sb[:, r2, :]),
                         inc="dve")
                    mask_done[buf] = cnt["dve"]
                    wait("tensor", "dve", cnt["dve"])
                else:
                    wait("tensor", "act", cnt["act"])
                # --- PV pair ---
                wait("tensor", "dve", vp_done[kj])
                for hh in range(2):
                    bank = 2 * hp + hh
                    if kj == 0:
                        wait_bank(bank)
                    h = 2 * hp + hh
                    emit("tensor",
                         lambda e, h2=hh, k2=kj, h3=h, b2=buf, bk=bank,
                         last=(kj == nkt - 1), first=(kj == 0):
                         nc.tensor.matmul(
                             P[bk][0:65, :],
                             vp_sb[:, k2, h3 * 65:(h3 + 1) * 65],
                             p_sb[:, b2, h2 * 512:(h2 + 1) * 512],
                             start=first, stop=last,
                             skip_group_check=True),
                         inc="pe" if hh == 1 else None)
                pvread_done[buf] = cnt["pe"]
                # --- interleaved chain slot ---
                if kj >= 2 and il:
                    il.pop(0)()
            # leftover chains
            while il:
                il.pop(0)()
            pv_stop = cnt["pe"]
            # ---------- normalization ----------
            qsl = slice(qt * TQ, (qt + 1) * TQ)
            wait("vector", "pe", pv_stop)
            wait("vector", "dve", memset_done)
            # den_sb reuse: previous den-out DMA must have read den_sb
            wait("vector", "r1", cnt["r1"])
            for hh in range(2):
                bank = 2 * hp + hh
                # bounce the denominator row through SBUF (plain DVE copy;
                # base-64 psum read is baseline-validated)
                emit("vector",
                     lambda e, bk=bank, h2=hh: nc.vector.tensor_copy(
                         den_sb[0:1, h2, :], P[bk][64:65, :]),
                     inc="dve" if hh == 1 else None)
            den_ready = cnt["dve"]
            # hop 1: den row to DRAM (contiguous)
            wait("sync", "dve", den_ready)
            wait("sync", "r2", cnt["r2"])   # den_dram slot reuse
            emit("sync",
                 lambda e, p2=hp: e.dma_start(
                     out=den_dram[p2:p2 + 1, :], in_=den_sb[0:1, :, :]),
                 inc="r1", inc_by=16)
            # hop 2: read back reshaped [128, 8] so q spreads over partitions
            wait("sync", "r1", cnt["r1"])
            def _rsrc(p2):
                a = den_dram[p2:p2 + 1, :]
                return bass.AP(tensor=a.tensor, offset=a.offset,
                               ap=[[8, 128], [1, 8]])
            emit("sync",
                 lambda e, p2=hp: e.dma_start(out=rsq_sb[:], in_=_rsrc(p2)),
                 inc="r2", inc_by=16)
            # reciprocal on 128 partitions x 8 elements (cheap)
            wait("vector", "r2", cnt["r2"])
            # rsq2 reuse: previous hop-3 DMA must have read rsq2
            wait("vector", "ro", cnt["ro"])
            emit("vector",
                 lambda e: nc.vector.reciprocal(rsq2_sb[:], rsq_sb[:]),
                 inc="dve")
            rec_done = cnt["dve"]
            # hop 3: reciprocals back to DRAM (contiguous layout again)
            wait("sync", "dve", rec_done)
            wait("sync", "rb", cnt["rb"])   # rec_dram slot reuse
            def _rdst(p2):
                a = rec_dram[p2:p2 + 1, :]
                return bass.AP(tensor=a.tensor, offset=a.offset,
                               ap=[[8, 128], [1, 8]])
            emit("sync",
                 lambda e, p2=hp: e.dma_start(out=_rdst(p2), in_=rsq2_sb[:]),
                 inc="ro", inc_by=16)
            # hop 4: broadcast across partitions
            wait("sync", "ro", cnt["ro"])
            for hh in range(2):
                def _bsrc(p2, h2):
                    a = rec_dram[p2:p2 + 1, h2 * 512:(h2 + 1) * 512]
                    return bass.AP(tensor=a.tensor, offset=a.offset,
                                   ap=[[0, 128], [1, 512]])
                emit("sync",
                     lambda e, p2=hp, h2=hh: e.dma_start(
                         out=rb_sb[:, h2, :], in_=_bsrc(p2, h2)),
                     inc="rb", inc_by=16)
            wait("vector", "rb", cnt["rb"])
            emit("vector",
                 lambda e, p2=hp, s=qsl: nc.vector.tensor_mul(
                     at_sb[0:64, p2, s], P[2 * p2][0:64, :], rb_sb[0:64, 0, :]))
            emit("vector",
                 lambda e, p2=hp, s=qsl: nc.vector.tensor_mul(
                     at_sb[64:128, p2, s], P[2 * p2 + 1][0:64, :],
                     rb_sb[64:128, 1, :]),
                 inc="dve")
            norm_done[(qt, hp)] = cnt["dve"]
            bank_free[2 * hp] = ("dve", cnt["dve"])
            bank_free[2 * hp + 1] = ("dve", cnt["dve"])

        for qt in range(NQT):
            # hp=0: interleave D(qt-1) into P2/P3, then B(qt+1) into P2/P3
            il0 = []
            if qt >= 1:
                for i, (tq, n) in enumerate(
                        [(t, n) for t in range(4 * (qt - 1), 4 * qt)
                         for n in range(2)]):
                    il0.append(lambda q2=qt - 1, t2=tq, n2=n, bk=2 + i % 2:
                               d_chain(q2, t2, n2, bk))
            if qt + 1 < NQT:
                for i, tt in enumerate(range(4 * (qt + 1), 4 * (qt + 2))):
                    il0.append(lambda t2=qt + 1, tt2=tt, bk=2 + i % 2:
                               b_chain(t2, tt2, bk))
            c_block(qt, 0, il0)
            # hp=1: interleave A(qt+1) into P0/P1
            il1 = []
            if qt + 1 < NQT:
                for i, m in enumerate([0, 2, 1, 3]):
                    il1.append(lambda t2=qt + 1, m2=m, bk=i % 2:
                               a_chain(t2, m2, bk))
            c_block(qt, 1, il1)

        # ---------- epilogue: D(3) ----------
        for i, (tq, n) in enumerate([(t, n) for t in range(12, 16)
                                     for n in range(2)]):
            d_chain(3, tq, n, i % 4)

        # final barrier on sync
        wait("sync", "od0", cnt["od0"])
        wait("sync", "od1", cnt["od1"])
        wait("sync", "rb", cnt["rb"])
        wait("sync", "r1", cnt["r1"])
        wait("sync", "r2", cnt["r2"])
        wait("sync", "ro", cnt["ro"])
        wait("sync", "act", cnt["act"])
        wait("sync", "dve", cnt["dve"])
        wait("sync", "pe", cnt["pe"])

        # ---------- emit per-engine programs ----------
        def runner(name):
            def _run(eng):
                for e_name, fn in sched:
                    if e_name == name:
                        fn(eng)
            return _run

        block.tensor(runner("tensor"))
        block.scalar(runner("scalar"))
        block.vector(runner("vector"))
        block.sync(runner("sync"))

    return nc


_NC_CACHE = {}
_RUN_KWARGS = {}
_LAST_RESULT = None


def _get_nc(causal: bool):
    if causal not in _NC_CACHE:
        _NC_CACHE[causal] = _build_nc(causal)
    return _NC_CACHE[causal]


def _host_inputs(x, Wqkv, Wout, cos, sin):
    import ml_dtypes
    bf16 = ml_dtypes.bfloat16
    kl = np.arange(128)[:, None]
    cc = np.arange(TQ)[None, :]
    dm_blocks = []
    for r in range(4):
        dmr = (128 * r + kl <= cc).astype(np.float32)
        dm_blocks.append(np.concatenate([dmr, dmr], axis=1))
    dm2 = np.concatenate(dm_blocks, axis=1).astype(bf16)  # [128, 4096]
    ct = np.ascontiguousarray(cos.T).astype(np.float32)   # [32, T]
    st = np.ascontiguousarray(sin.T).astype(np.float32)
    cosr = np.tile(ct, (4, 1)).astype(bf16)
    sinsg = np.concatenate([-st, st, -st, st], axis=0).astype(bf16)
    Wq, Wk, Wv = Wqkv[:, 0:D], Wqkv[:, D:2 * D], Wqkv[:, 2 * D:3 * D]
    in_maps = []
    for core in range(8):
        b, g = divmod(core, NG)
        hs = slice(g * HPC * DH, (g + 1) * HPC * DH)
        in_maps.append({
            "xT": np.ascontiguousarray(x[b].T).astype(bf16),
            "wqk": np.concatenate([Wq[:, hs], Wk[:, hs]], axis=1).astype(bf16),
            "wv": np.ascontiguousarray(Wv[:, hs]).astype(bf16),
            "wo": np.ascontiguousarray(Wout[hs, :]).astype(bf16),
            "cosr": cosr,
            "sinsg": sinsg,
            "dm2": dm2,
        })
    return in_maps


def kernel(x, Wqkv, Wout, cos, sin, mask):
    import sys
    if "/opt/trn_rl_repo" not in sys.path:
        sys.path.insert(0, "/opt/trn_rl_repo")
    from concourse.bass_utils import run_bass_kernel_spmd

    x = np.asarray(x)
    mask = np.asarray(mask)
    m2 = mask.reshape(T, T)
    causal = bool(np.array_equal(m2, np.tril(np.ones((T, T), dtype=bool))))
    if not causal:
        assert m2.all(), "only causal or all-ones masks supported"

    in_maps = _host_inputs(x, np.asarray(Wqkv), np.asarray(Wout),
                           np.asarray(cos), np.asarray(sin))
    nc = _get_nc(causal)
    res = run_bass_kernel_spmd(nc, in_maps, list(range(8)), **_RUN_KWARGS)
    global _LAST_RESULT
    _LAST_RESULT = res
    outs = [np.asarray(r["out"], dtype=np.float32) for r in res.results]
    return np.stack([outs[0] + outs[1] + outs[2] + outs[3],
                     outs[4] + outs[5] + outs[6] + outs[7]])


# revision 3
# speedup vs baseline: 1.4982x; 1.4982x over previous
"""Distributed causal MHA + RoPE kernel for 8 TRN2 NeuronCores (raw Bass).

Rewrite of the staged baseline with a qt-major software pipeline:

  - 8 cores = 2 batches x 4 head-groups (4 heads each); each core emits a
    partial out-projection (its heads' rows of Wout) in bf16; host sums.
  - Per q-block (512 rows) and head pair: S^T tiles via K=64 matmuls with
    row-group packing (head0 rows 0-63, head1 rows 64-127 issued
    back-to-back so they run concurrently in disjoint PE row groups),
    exp on ACT over [128,1024] spanning both heads' tiles (psum f32,
    double-buffered in 4 banks), causal diag mask on DVE, PV accumulated
    per head into dedicated psum banks with a ones-column for the
    softmax denominators.
  - RoPE: qk*cos + swap(qk)*sign_sin where swap is 4 SBUF->SBUF DMA
    block copies and sign_sin has the -/+ pattern baked in host-side;
    3 big DVE ops per (block, chunk).
  - Denominator reciprocals via DVE stream-transpose trick: transpose
    [32,512] so the 512 denoms land 16-per-partition, reciprocal on the
    strided view, transpose back; broadcast via DRAM round-trip DMA.
  - Projection (A), V (B) and out-projection (D) matmul chains are
    interleaved into the attention stream and borrow the PV psum banks
    of the opposite head-pair while those are idle.
"""

import numpy as np

B, T, D, H, DH = 2, 2048, 1024, 16, 64
HPC = 4
NG = 4
TQ = 512
NQT = T // TQ      # 4
NKT = T // 128     # 16
KC = D // 128      # 8


def _build_nc(causal: bool):
    import concourse.bass as bass
    import concourse.mybir as mybir
    from contextlib import ExitStack

    dt = mybir.dt
    f32, bf16 = dt.float32, dt.bfloat16
    AF = mybir.ActivationFunctionType
    nc = bass.Bass()

    xT = nc.declare_dram_parameter("xT", [D, T], bf16, isOutput=False)
    wqk = nc.declare_dram_parameter("wqk", [D, 512], bf16, isOutput=False)
    wv = nc.declare_dram_parameter("wv", [D, 256], bf16, isOutput=False)
    wo = nc.declare_dram_parameter("wo", [256, D], bf16, isOutput=False)
    cosr = nc.declare_dram_parameter("cosr", [128, T], bf16, isOutput=False)
    sinsg = nc.declare_dram_parameter("sinsg", [128, T], bf16, isOutput=False)
    dm2 = nc.declare_dram_parameter("dm2", [128, 4096], bf16, isOutput=False)
    out = nc.declare_dram_parameter("out", [T, D], bf16, isOutput=True)
    den_dram = nc.dram_tensor("den_dram", [2, 1024], f32)
    rec_dram = nc.dram_tensor("rec_dram", [2, 1024], f32)

    ctx = ExitStack()
    with ctx:
        sb = lambda name, shape, dtype: ctx.enter_context(
            nc.sbuf_tensor(name, shape, dtype))

        xt_sb = sb("xt_sb", [128, KC, T], bf16)
        wqk_sb = sb("wqk_sb", [128, KC, 512], bf16)
        wv_sb = sb("wv_sb", [128, KC, 256], bf16)
        wo_sb = sb("wo_sb", [128, 2, D], bf16)
        cos_sb = sb("cos_sb", [128, T], bf16)
        sin_sb = sb("sin_sb", [128, T], bf16)
        dm_sb = sb("dm_sb", [128, 4, 1024], bf16)
        qk_sb = sb("qk_sb", [128, 4, T], bf16)
        qsw_sb = sb("qsw_sb", [128, 4, T], bf16)
        qkr_sb = sb("qkr_sb", [128, 2, T], bf16)
        # k stationary, zero-padded to K=128: kz[:,hh,mk,:] holds head hh's
        # rotated k rows in its own 64-partition half, zeros elsewhere, so
        # every S matmul is a full-array (128,128)-tile op - no PE tiling
        # mode switches mid-stream.
        kz_sb = sb("kz_sb", [128, 2, 2, T], bf16)
        rtmp_sb = sb("rtmp_sb", [128, 2, TQ], bf16)
        vp_sb = sb("vp_sb", [128, NKT, HPC * 65], bf16)
        p_sb = sb("p_sb", [128, 2, 1024], bf16)
        at_sb = sb("at_sb", [128, 2, T], bf16)
        ob_sb = sb("ob_sb", [128, 2, 512], bf16)
        den_sb = sb("den_sb", [1, 2, 512], f32)
        ocp_sb = sb("ocp_sb", [128, 2, 512], bf16)
        rsq_sb = sb("rsq_sb", [128, 8], f32)
        rsq2_sb = sb("rsq2_sb", [128, 8], f32)
        rb_sb = sb("rb_sb", [128, 2, 512], f32)

        pS = ctx.enter_context(nc.psum_tensor("pS", [128, 2048], f32))
        P = [ctx.enter_context(nc.psum_tensor(f"P{i}", [128, 512], f32))
             for i in range(4)]

        in_sems = ["iw", "ic", "isn", "iv", "io", "idm",
                   "ix0", "ix1", "ix2", "ix3"]
        sw_sems = [f"sw{i}" for i in range(16)]
        sem_names = (["pe", "act", "dve", "r1", "r2", "ro", "rb",
                      "od0", "od1"] + in_sems + sw_sems)
        sems = {n: ctx.enter_context(nc.semaphore(f"s_{n}")) for n in sem_names}
        block = ctx.enter_context(nc.Block())

        # ---------- schedule machinery ----------
        sched = []
        cnt = {n: 0 for n in sem_names}
        last_wait = {}

        def wait(eng, sem, val):
            if val <= 0:
                return
            key = (eng, sem)
            if last_wait.get(key, -1) >= val:
                return
            last_wait[key] = val
            sched.append((eng, lambda e, s=sems[sem], v=val: e.wait_ge(s, v)))

        def emit(eng, fn, inc=None, inc_by=1):
            if inc is None:
                sched.append((eng, fn))
            else:
                s = sems[inc]
                sched.append((eng, lambda e, f=fn, ss=s, ib=inc_by:
                              f(e).then_inc(ss, ib)))
                cnt[inc] += inc_by

        # ---------- state tracking ----------
        rope_done = {}       # (tb, m) -> dve cnt
        vp_done = {}         # tt -> dve cnt
        exp_done = {}        # buf -> act cnt
        mask_done = {}       # buf -> dve cnt
        pvread_done = {}     # buf -> pe cnt (PV pair that read p_sb[buf])
        bank_free = {i: None for i in range(4)}   # bank -> (sem, cnt)
        norm_done = {}       # (qt, hp) -> dve cnt
        od_cnt = {0: 0, 1: 0}
        ob_slot = [0]
        swap_grp = [0]

        def wait_bank(bank):
            g = bank_free[bank]
            if g is not None:
                wait("tensor", g[0], g[1])

        # ---------- input DMAs ----------
        def dma_in(sem, dst, src):
            emit("sync", lambda e, d=dst, s=src: e.dma_start(out=d, in_=s),
                 inc=sem, inc_by=16)

        xre = xT.rearrange("(c p) t -> p c t", p=128)
        dma_in("iw", wqk_sb[:], wqk.rearrange("(c p) m -> p c m", p=128))
        dma_in("ix0", xt_sb[:, :, 0:TQ], xre[:, :, 0:TQ])
        dma_in("ic", cos_sb[:], cosr[:])
        dma_in("isn", sin_sb[:], sinsg[:])
        dma_in("iv", wv_sb[:], wv.rearrange("(c p) m -> p c m", p=128))
        dma_in("ix1", xt_sb[:, :, TQ:2 * TQ], xre[:, :, TQ:2 * TQ])
        dma_in("io", wo_sb[:], wo.rearrange("(h p) n -> p h n", p=128))
        dma_in("idm", dm_sb[:], dm2.rearrange("p (r n) -> p r n", r=4))
        dma_in("ix2", xt_sb[:, :, 2 * TQ:3 * TQ], xre[:, :, 2 * TQ:3 * TQ])
        dma_in("ix3", xt_sb[:, :, 3 * TQ:4 * TQ], xre[:, :, 3 * TQ:4 * TQ])

        # ---------- memsets ----------
        vpv = vp_sb.rearrange("p n (h m) -> p n h m", m=65)
        emit("vector", lambda e: nc.vector.memset(vpv[:, :, :, 64:65], 1.0),
             inc="dve")
        emit("vector", lambda e: nc.vector.memset(kz_sb[:], 0.0), inc="dve")
        memset_done = cnt["dve"]

        # ---------- chains ----------
        def a_chain(tb, m, bank):
            """qkT projection chunk m of t-block tb + rope pipeline."""
            sl = slice(tb * TQ, (tb + 1) * TQ)
            wait("tensor", "iw", 16)
            wait("tensor", f"ix{tb}", 16)
            wait_bank(bank)
            for c in range(KC):
                emit("tensor",
                     lambda e, cc=c, mm=m, s=sl: nc.tensor.matmul(
                         P[bank][:], wqk_sb[:, cc, mm * 128:(mm + 1) * 128],
                         xt_sb[:, cc, s],
                         start=(cc == 0), stop=(cc == KC - 1)),
                     inc="pe" if c == KC - 1 else None)
            pe_after = cnt["pe"]
            wait("scalar", "pe", pe_after)
            emit("scalar",
                 lambda e, mm=m, s=sl: nc.scalar.copy(
                     qk_sb[:, mm, s], P[bank][:]),
                 inc="act")
            copy_cnt = cnt["act"]
            bank_free[bank] = ("act", copy_cnt)
            # swap DMAs
            g = swap_grp[0]
            swap_grp[0] += 1
            gsem = f"sw{g}"
            wait("sync", "act", copy_cnt)
            for dst0, src0 in ((0, 32), (32, 0), (64, 96), (96, 64)):
                emit("sync",
                     lambda e, d0=dst0, s0=src0, mm=m, s=sl: e.dma_start(
                         out=qsw_sb[d0:d0 + 32, mm, s],
                         in_=qk_sb[s0:s0 + 32, mm, s]),
                     inc=gsem, inc_by=16)
            # rope on DVE
            wait("vector", gsem, 64)
            wait("vector", "ic", 16)
            wait("vector", "isn", 16)
            emit("vector",
                 lambda e, mm=m, s=sl: nc.vector.tensor_mul(
                     rtmp_sb[:, 0, :], qk_sb[:, mm, s], cos_sb[:, s]))
            emit("vector",
                 lambda e, mm=m, s=sl: nc.vector.tensor_mul(
                     rtmp_sb[:, 1, :], qsw_sb[:, mm, s], sin_sb[:, s]))
            if m < 2:
                emit("vector",
                     lambda e, mm=m, s=sl: nc.vector.tensor_add(
                         qkr_sb[:, mm, s], rtmp_sb[:, 0, :], rtmp_sb[:, 1, :]),
                     inc="dve")
            else:
                emit("vector",
                     lambda e, mk=m - 2, s=sl: nc.vector.tensor_add(
                         kz_sb[0:64, 0, mk, s], rtmp_sb[0:64, 0, :],
                         rtmp_sb[0:64, 1, :]))
                emit("vector",
                     lambda e, mk=m - 2, s=sl: nc.vector.tensor_add(
                         kz_sb[64:128, 1, mk, s], rtmp_sb[64:128, 0, :],
                         rtmp_sb[64:128, 1, :]),
                     inc="dve")
            rope_done[(tb, m)] = cnt["dve"]

        def b_chain(tb, tt, bank):
            """V natural chunk for ktile tt (128 rows)."""
            wait("tensor", "iv", 16)
            wait("tensor", f"ix{tb}", 16)
            wait_bank(bank)
            for c in range(KC):
                emit("tensor",
                     lambda e, cc=c, t2=tt: nc.tensor.matmul(
                         P[bank][:, 0:256],
                         xt_sb[:, cc, t2 * 128:(t2 + 1) * 128],
                         wv_sb[:, cc, :],
                         start=(cc == 0), stop=(cc == KC - 1)),
                     inc="pe" if c == KC - 1 else None)
            wait("vector", "pe", cnt["pe"])
            wait("vector", "dve", memset_done)
            emit("vector",
                 lambda e, t2=tt: nc.vector.tensor_copy(
                     vpv[:, t2, :, 0:64],
                     P[bank][:, 0:256].rearrange("p (h m) -> p h m", m=64)),
                 inc="dve")
            vp_done[tt] = cnt["dve"]
            bank_free[bank] = ("dve", cnt["dve"])

        def d_chain(qt, tq, n, bank):
            """out-projection tile: ttile tq (128 rows), n-chunk (512 cols)."""
            wait("tensor", "io", 16)
            wait("tensor", "dve", norm_done[(qt, 0)])
            wait("tensor", "dve", norm_done[(qt, 1)])
            wait_bank(bank)
            for hp in range(2):
                emit("tensor",
                     lambda e, h2=hp, t2=tq, n2=n: nc.tensor.matmul(
                         P[bank][:],
                         at_sb[:, h2, t2 * 128:(t2 + 1) * 128],
                         wo_sb[:, h2, n2 * 512:(n2 + 1) * 512],
                         start=(h2 == 0), stop=(h2 == 1)),
                     inc="pe" if hp == 1 else None)
            wait("vector", "pe", cnt["pe"])
            slot = ob_slot[0]
            ob_slot[0] ^= 1
            # ob slot reuse: wait for previous out-DMA from this slot
            wait("vector", f"od{slot}", od_cnt[slot])
            emit("vector",
                 lambda e, s=slot: nc.vector.tensor_copy(
                     ob_sb[:, s, :], P[bank][:]),
                 inc="dve")
            bank_free[bank] = ("dve", cnt["dve"])
            wait("sync", "dve", cnt["dve"])
            emit("sync",
                 lambda e, t2=tq, n2=n, s=slot: e.dma_start(
                     out=out[t2 * 128:(t2 + 1) * 128, n2 * 512:(n2 + 1) * 512],
                     in_=ob_sb[:, s, :]),
                 inc=f"od{slot}", inc_by=16)
            od_cnt[slot] = cnt[f"od{slot}"]

        # ---------- prologue ----------
        a_chain(0, 0, 2)
        a_chain(0, 2, 3)
        b_chain(0, 0, 2)
        b_chain(0, 1, 3)
        a_chain(0, 1, 2)
        a_chain(0, 3, 3)
        b_chain(0, 2, 2)
        b_chain(0, 3, 3)

        # ---------- attention + interleaved chains ----------
        scale = 0.125

        def c_block(qt, hp, interleave):
            """Attention for (qt, hp). interleave: list of chain thunks."""
            nkt = 4 * (qt + 1) if causal else NKT
            il = list(interleave)
            for kj in range(nkt):
                buf = kj % 2
                # --- S pair ---
                if exp_done.get(buf) is not None:
                    wait("tensor", "act", exp_done[buf])
                wait("tensor", "dve", rope_done[(kj // 4, 2 + hp)])
                wait("tensor", "dve", rope_done[(qt, hp)])
                for hh in range(2):
                    emit("tensor",
                         lambda e, h2=hh, k2=kj, q2=qt, p2=hp, b2=buf:
                         nc.tensor.matmul(
                             pS[:, b2 * 1024 + h2 * 512:
                                b2 * 1024 + (h2 + 1) * 512],
                             kz_sb[:, h2, p2, k2 * 128:(k2 + 1) * 128],
                             qkr_sb[:, p2, q2 * TQ:(q2 + 1) * TQ],
                             start=True, stop=True),
                         inc="pe" if hh == 1 else None)
                s_cnt = cnt["pe"]
                # --- exp ---
                wait("scalar", "pe", s_cnt)
                if mask_done.get(buf) is not None:
                    wait("scalar", "dve", mask_done[buf])
                if pvread_done.get(buf) is not None:
                    wait("scalar", "pe", pvread_done[buf])
                emit("scalar",
                     lambda e, b2=buf: nc.scalar.activation(
                         p_sb[:, b2, :], pS[:, b2 * 1024:(b2 + 1) * 1024],
                         AF.Exp, scale=scale),
                     inc="act")
                exp_done[buf] = cnt["act"]
                # --- diag mask ---
                r = kj - 4 * qt
                diag = causal and r >= 0
                if diag:
                    wait("vector", "act", cnt["act"])
                    wait("vector", "idm", 16)
                    emit("vector",
                         lambda e, b2=buf, r2=r: nc.vector.tensor_mul(
                             p_sb[:, b2, :], p_sb[:, b2, :], dm_sb[:, r2, :]),
                         inc="dve")
                    mask_done[buf] = cnt["dve"]
                    wait("tensor", "dve", cnt["dve"])
                else:
                    wait("tensor", "act", cnt["act"])
                # --- PV pair ---
                wait("tensor", "dve", vp_done[kj])
                for hh in range(2):
                    bank = 2 * hp + hh
                    if kj == 0:
                        wait_bank(bank)
                    h = 2 * hp + hh
                    emit("tensor",
                         lambda e, h2=hh, k2=kj, h3=h, b2=buf, bk=bank,
                         last=(kj == nkt - 1), first=(kj == 0):
                         nc.tensor.matmul(
                             P[bk][0:65, :],
                             vp_sb[:, k2, h3 * 65:(h3 + 1) * 65],
                             p_sb[:, b2, h2 * 512:(h2 + 1) * 512],
                             start=first, stop=last,
                             skip_group_check=True),
                         inc="pe" if hh == 1 else None)
                pvread_done[buf] = cnt["pe"]
                # --- interleaved chain slot ---
                if kj >= 2 and il:
                    il.pop(0)()
            # leftover chains
            while il:
                il.pop(0)()
            pv_stop = cnt["pe"]
            # ---------- normalization ----------
            qsl = slice(qt * TQ, (qt + 1) * TQ)
            wait("vector", "pe", pv_stop)
            wait("vector", "dve", memset_done)
            # den_sb reuse: previous reshape DMA must have read den_sb
            wait("vector", "r2", cnt["r2"])
            for hh in range(2):
                bank = 2 * hp + hh
                # bounce the denominator row through SBUF (plain DVE copy;
                # base-64 psum read is baseline-validated)
                emit("vector",
                     lambda e, bk=bank, h2=hh: nc.vector.tensor_copy(
                         den_sb[0:1, h2, :], P[bk][64:65, :]))
                # evacuate this head's o rows to SBUF so the psum bank is
                # released immediately - the normalize multiply then runs
                # entirely off SBUF, off the PV critical path.
                emit("vector",
                     lambda e, bk=bank, h2=hh: nc.vector.tensor_copy(
                         ocp_sb[h2 * 64:(h2 + 1) * 64, hp, :],
                         P[bk][0:64, :]),
                     inc="dve")
                bank_free[bank] = ("dve", cnt["dve"])
            den_ready = cnt["dve"]
            # hop 1+2 fused: SBUF->SBUF reshape DMA, q spreads over partitions
            wait("sync", "dve", den_ready)
            emit("sync",
                 lambda e: e.dma_start(out=rsq_sb[:], in_=den_sb[0:1, :, :]),
                 inc="r2", inc_by=16)
            # reciprocal on 128 partitions x 8 elements (cheap)
            wait("vector", "r2", cnt["r2"])
            # rsq2 reuse: previous hop-3 DMA must have read rsq2
            wait("vector", "ro", cnt["ro"])
            emit("vector",
                 lambda e: nc.vector.reciprocal(rsq2_sb[:], rsq_sb[:]),
                 inc="dve")
            rec_done = cnt["dve"]
            # hop 3: reciprocals back to DRAM (contiguous layout again)
            wait("sync", "dve", rec_done)
            wait("sync", "rb", cnt["rb"])   # rec_dram slot reuse
            def _rdst(p2):
                a = rec_dram[p2:p2 + 1, :]
                return bass.AP(tensor=a.tensor, offset=a.offset,
                               ap=[[8, 128], [1, 8]])
            emit("sync",
                 lambda e, p2=hp: e.dma_start(out=_rdst(p2), in_=rsq2_sb[:]),
                 inc="ro", inc_by=16)
            # hop 4: broadcast across partitions
            wait("sync", "ro", cnt["ro"])
            for hh in range(2):
                def _bsrc(p2, h2):
                    a = rec_dram[p2:p2 + 1, h2 * 512:(h2 + 1) * 512]
                    return bass.AP(tensor=a.tensor, offset=a.offset,
                                   ap=[[0, 128], [1, 512]])
                emit("sync",
                     lambda e, p2=hp, h2=hh: e.dma_start(
                         out=rb_sb[:, h2, :], in_=_bsrc(p2, h2)),
                     inc="rb", inc_by=16)
            wait("vector", "rb", cnt["rb"])
            emit("vector",
                 lambda e, p2=hp, s=qsl: nc.vector.tensor_mul(
                     at_sb[0:64, p2, s], ocp_sb[0:64, p2, :],
                     rb_sb[0:64, 0, :]))
            emit("vector",
                 lambda e, p2=hp, s=qsl: nc.vector.tensor_mul(
                     at_sb[64:128, p2, s], ocp_sb[64:128, p2, :],
                     rb_sb[64:128, 1, :]),
                 inc="dve")
            norm_done[(qt, hp)] = cnt["dve"]

        for qt in range(NQT):
            # hp=0: interleave D(qt-1) into P2/P3, then B(qt+1) into P2/P3
            il0 = []
            if qt >= 1:
                for i, (tq, n) in enumerate(
                        [(t, n) for t in range(4 * (qt - 1), 4 * qt)
                         for n in range(2)]):
                    il0.append(lambda q2=qt - 1, t2=tq, n2=n, bk=2 + i % 2:
                               d_chain(q2, t2, n2, bk))
            if qt + 1 < NQT:
                for i, tt in enumerate(range(4 * (qt + 1), 4 * (qt + 2))):
                    il0.append(lambda t2=qt + 1, tt2=tt, bk=2 + i % 2:
                               b_chain(t2, tt2, bk))
            c_block(qt, 0, il0)
            # hp=1: interleave A(qt+1) into P0/P1
            il1 = []
            if qt + 1 < NQT:
                for i, m in enumerate([0, 2, 1, 3]):
                    il1.append(lambda t2=qt + 1, m2=m, bk=i % 2:
                               a_chain(t2, m2, bk))
            c_block(qt, 1, il1)

        # ---------- epilogue: D(3) ----------
        for i, (tq, n) in enumerate([(t, n) for t in range(12, 16)
                                     for n in range(2)]):
            d_chain(3, tq, n, i % 4)

        # final barrier on sync
        wait("sync", "od0", cnt["od0"])
        wait("sync", "od1", cnt["od1"])
        wait("sync", "rb", cnt["rb"])
        wait("sync", "r1", cnt["r1"])
        wait("sync", "r2", cnt["r2"])
        wait("sync", "ro", cnt["ro"])
        wait("sync", "act", cnt["act"])
        wait("sync", "dve", cnt["dve"])
        wait("sync", "pe", cnt["pe"])

        # ---------- emit per-engine programs ----------
        def runner(name):
            def _run(eng):
                for e_name, fn in sched:
                    if e_name == name:
                        fn(eng)
            return _run

        block.tensor(runner("tensor"))
        block.scalar(runner("scalar"))
        block.vector(runner("vector"))
        block.sync(runner("sync"))

    return nc


_NC_CACHE = {}
_RUN_KWARGS = {}
_LAST_RESULT = None


def _get_nc(causal: bool):
    if causal not in _NC_CACHE:
        _NC_CACHE[causal] = _build_nc(causal)
    return _NC_CACHE[causal]


def _host_inputs(x, Wqkv, Wout, cos, sin):
    import ml_dtypes
    bf16 = ml_dtypes.bfloat16
    kl = np.arange(128)[:, None]
    cc = np.arange(TQ)[None, :]
    dm_blocks = []
    for r in range(4):
        dmr = (128 * r + kl <= cc).astype(np.float32)
        dm_blocks.append(np.concatenate([dmr, dmr], axis=1))
    dm2 = np.concatenate(dm_blocks, axis=1).astype(bf16)  # [128, 4096]
    ct = np.ascontiguousarray(cos.T).astype(np.float32)   # [32, T]
    st = np.ascontiguousarray(sin.T).astype(np.float32)
    cosr = np.tile(ct, (4, 1)).astype(bf16)
    sinsg = np.concatenate([-st, st, -st, st], axis=0).astype(bf16)
    Wq, Wk, Wv = Wqkv[:, 0:D], Wqkv[:, D:2 * D], Wqkv[:, 2 * D:3 * D]
    in_maps = []
    for core in range(8):
        b, g = divmod(core, NG)
        hs = slice(g * HPC * DH, (g + 1) * HPC * DH)
        in_maps.append({
            "xT": np.ascontiguousarray(x[b].T).astype(bf16),
            "wqk": np.concatenate([Wq[:, hs], Wk[:, hs]], axis=1).astype(bf16),
            "wv": np.ascontiguousarray(Wv[:, hs]).astype(bf16),
            "wo": np.ascontiguousarray(Wout[hs, :]).astype(bf16),
            "cosr": cosr,
            "sinsg": sinsg,
            "dm2": dm2,
        })
    return in_maps


def kernel(x, Wqkv, Wout, cos, sin, mask):
    import sys
    if "/opt/trn_rl_repo" not in sys.path:
        sys.path.insert(0, "/opt/trn_rl_repo")
    from concourse.bass_utils import run_bass_kernel_spmd

    x = np.asarray(x)
    mask = np.asarray(mask)
    m2 = mask.reshape(T, T)
    causal = bool(np.array_equal(m2, np.tril(np.ones((T, T), dtype=bool))))
    if not causal:
        assert m2.all(), "only causal or all-ones masks supported"

    in_maps = _host_inputs(x, np.asarray(Wqkv), np.asarray(Wout),
                           np.asarray(cos), np.asarray(sin))
    nc = _get_nc(causal)
    res = run_bass_kernel_spmd(nc, in_maps, list(range(8)), **_RUN_KWARGS)
    global _LAST_RESULT
    _LAST_RESULT = res
    outs = [np.asarray(r["out"], dtype=np.float32) for r in res.results]
    return np.stack([outs[0] + outs[1] + outs[2] + outs[3],
                     outs[4] + outs[5] + outs[6] + outs[7]])
